# revision 12
# baseline (speedup 1.0000x reference)
"""Location-dependent 3D conv (AsymConv) on 8 TRN2 NeuronCores.

Math (per output voxel):
    out[b, 0, x, y, z] = sum_{i,j,l in 0..2} Xp[b, x+i, y+j, z+l] * W[x, y, z, (i*3+j)*3+l]
with Xp = edge-padded X by 1 plane on each spatial side.

Strategy (128-partition z-split layout):
  - Shard the x axis across cores (12 planes each, halo 14) -> no inter-core
    communication.
  - Per core, partition rows r = (zq, y) with zq in 0..3 a z-quarter and y the
    full 96: 384 rows = exactly 3 "slots" of 128 partitions, kept as one
    merged SBUF tile so every DVE op uses all 128 lanes (the old y-partition
    layout only used 96). Batch b lives in the free dims; W (which has no
    batch dim) is read via a stride-0 broadcast AP, so each W element moves
    from HBM exactly once: per-core traffic 5.97 MB W + 0.84 MB X + 0.44 MB
    out ~= 7.3 MB (~20 us at 360 GB/s) vs ~8 MB before.
  - Per partition the host ships a halo slab [b, y' 3, x 14, zw 26]: all 27
    taps are then free-dim offsets (j, i, l) into the slab; no partition-dim
    shifts anywhere.
  - Products run on the Vector engine (tensor_tensor, fp16 2x mode). Taps
    with l in {0, 2} of each (i, j) fuse into one op via an overlapping
    stride-2 AP dim; l == 1 starts are 2B-misaligned (breaks 2x), so those
    taps read a z-shifted slab copy built on the ScalarE. 9 pair ops + 9
    single ops. A few pair units can be offloaded to the otherwise-idle
    GpSimd (Pool) engine to shave the DVE critical path (ASYM_POOL).
  - 27-term accumulation on the TensorEngine: identity[128x128] matmuls into
    4 PSUM chunks of 432 fp32 columns (512-column moving limit, 2KB banks).
  - W moves as 7 grouped DMAs (4 taps each, 2304B descriptors) in consumption
    order; X slabs land first (3 slot DMAs); drain PSUM -> fp16 -> HBM.
"""

import os

import numpy as np

# ---- problem constants (hardcoded per harness rules) ----
B = 2
D = 96  # Dx = Dy = Dz
KSZ = 3
NTAP = KSZ**3  # 27
NCORES = 8
XS = D // NCORES  # 12 x-planes per core
XH = XS + 2  # with halo
NSLOT = 3  # partition tile slots: 384 rows / 128
ZQ = 4  # z quarters
ZW = D // ZQ  # 24 output z per quarter
ZWH = ZW + 2  # z window incl halo
NCOL = NSLOT * B * XS * ZW  # 1728 psum columns
PCH = 4  # psum chunks
CCH = NCOL // PCH  # 432 columns per chunk

F16 = np.float16
LAST_RESULT = None  # BassKernelResults of the most recent run (for test.py)

_GRAPH_CACHE = {}

N_WARMUP = int(os.environ.get("ASYM_WARMUP", "0"))
N_POOL = int(os.environ.get("ASYM_POOL", "0"))  # taps on GpSimd (SBUF-port contention: slows concurrent DVE ops ~4x, keep 0)

# taps with l != 1 are 4B-aligned in the base slab; issue them first so the
# ScalarE z-shift copies (needed by l == 1 taps) are off the critical path.
# Within the l != 1 block consecutive entries are the (l=0, l=2) taps of one
# (i, j), consumed as one fused DVE op.
TAP_ORDER = [
    t
    for lgroup in (False, True)
    for j in range(KSZ)
    for t in range(NTAP)
    if (t % 3 == 1) == lgroup and (t // 3) % 3 == j
]
# W DMA groups (consumption order), 4 taps per transfer (2304B descriptors)
WG_BOUNDS = [0, 2, 4, 8, 12, 16, 20, 24, 27]


def _build_graph():
    """Build (and cache) the per-core Bass graph. Same graph for all 8 cores."""
    key = ("nc", N_WARMUP, N_POOL)
    if key in _GRAPH_CACHE:
        return _GRAPH_CACHE[key]

    from concourse import bacc
    from concourse import bass as _bass
    import concourse.mybir as mybir
    from concourse.tile import TileContext

    f16 = mybir.dt.float16
    f32 = mybir.dt.float32
    MUL = mybir.AluOpType.mult

    nc = bacc.Bacc("TRN2", target_bir_lowering=False, debug=False, num_devices=NCORES)

    xs_d = nc.dram_tensor(
        "xslab", [128, NSLOT, B, KSZ, XH, ZWH], f16, kind="ExternalInput"
    )
    w_d = nc.dram_tensor("w", [128, NTAP, NSLOT, XS, ZW], f16, kind="ExternalInput")
    id_d = nc.dram_tensor("ident", [128, 128], f16, kind="ExternalInput")
    out_d = nc.dram_tensor("out", [128, NCOL], f16, kind="ExternalOutput")

    with TileContext(nc) as tc:
        with (
            tc.tile_pool(name="xp", bufs=1) as xpool,
            tc.tile_pool(name="wp", bufs=1) as wpool,
            tc.tile_pool(name="pp", bufs=4) as ppool,
            tc.tile_pool(name="psp", bufs=1, space="PSUM") as pspool,
        ):
            # ---- DMA in: X slabs first, first W group in parallel on the
            # ACT ring, then the W flood in consumption order ----
            xslab = xpool.tile(
                [128, NSLOT, B, KSZ, XH, ZWH], f16, name="xslab", tag="xslab"
            )
            w_tiles = []  # per group
            for g in range(len(WG_BOUNDS) - 1):
                t0, t1 = WG_BOUNDS[g], WG_BOUNDS[g + 1]
                wg = wpool.tile(
                    [128, t1 - t0, NSLOT, XS, ZW], f16, name=f"w_{g}", tag=f"w_{g}"
                )
                w_tiles.append(wg)

            # Input DMAs lead on the SP ring (earliest post-preamble issue
            # point); the slab moves as ONE 128-descriptor transfer, W in
            # tap-major layout (contiguous G*1728B per-partition runs). The
            # serialized DIRECT2D descriptor-generation cost (~1us per
            # dma_start on the issuing sequencer) is split across the SP and
            # otherwise-idle GpSimd rings.
            nc.sync.dma_start(out=xslab[:], in_=xs_d.ap())
            nc.sync.dma_start(out=w_tiles[0][:], in_=w_d.ap()[:, 0:2])
            id_t = xpool.tile([128, 128], f16, name="id_t", tag="id_t")
            nc.scalar.dma_start(out=id_t[:], in_=id_d.ap())
            for g in range(1, len(WG_BOUNDS) - 1):
                t0, t1 = WG_BOUNDS[g], WG_BOUNDS[g + 1]
                q = nc.sync if g % 2 else nc.gpsimd
                q.dma_start(out=w_tiles[g][:], in_=w_d.ap()[:, t0:t1])

            # ---- z-shifted slab for l == 1 taps (2B-aligned reads) ----
            xz = xpool.tile(
                [128, NSLOT, B, KSZ, XH, ZWH - 1], f16, name="xz", tag="xz"
            )
            for s in range(NSLOT):
                for b in range(B):
                    nc.scalar.copy(
                        out=xz[:, s, b], in_=xslab[:, s, b, :, :, 1:ZWH]
                    )

            # ---- PSUM accumulators ----
            psums = [
                pspool.tile([128, CCH], f32, name=f"ps_{ci}", tag=f"ps_{ci}")
                for ci in range(PCH)
            ]

            if N_WARMUP:
                dummy = ppool.tile([128, CCH], f16, name="warm", tag="warm", bufs=1)
                nc.vector.memset(dummy[:], 0.0)
                ps_w = pspool.tile([128, CCH], f32, name="ps_warm", tag="ps_warm")
                for _ in range(N_WARMUP):
                    nc.tensor.matmul(ps_w[:], id_t[:], dummy[:], start=True, stop=True)

            # ---- product + accumulate stream ----
            # walrus limits every engine to 3 free AP dims, so each tap is
            # one op: in0 [slot*b (fused), x, z], in1 W b-broadcast, out dense.
            # N_POOL taps run on the otherwise-idle GpSimd engine (no 2x mode
            # there, so they can read l==1 straight from the base slab).
            pool_set = set()
            if N_POOL:
                step = NTAP / N_POOL
                pool_set = {int(step * (n + 0.5)) for n in range(N_POOL)}

            acc_cnt = 0  # taps accumulated so far (start/stop flags)

            def mm_consume(prod):
                nonlocal acc_cnt
                pbase = prod[:]
                for ci in range(PCH):
                    rhs = _bass.AP(
                        pbase.tensor,
                        pbase.offset + ci * CCH,
                        [pbase.ap[0], [1, CCH]],
                    )
                    nc.tensor.matmul(
                        psums[ci][:],
                        id_t[:],
                        rhs,
                        start=(acc_cnt == 0),
                        stop=(acc_cnt == NTAP - 1),
                    )
                acc_cnt += 1

            def wg_of(wi):
                for g in range(len(WG_BOUNDS) - 1):
                    if WG_BOUNDS[g] <= wi < WG_BOUNDS[g + 1]:
                        return w_tiles[g], wi - WG_BOUNDS[g]
                raise AssertionError(wi)

            for wi in range(NTAP):
                t = TAP_ORDER[wi]
                i, j, l = t // 9, (t // 3) % 3, t % 3
                wg, kk = wg_of(wi)
                on_pool = wi in pool_set
                if l == 1 and not on_pool:
                    in0 = xz[:, :, :, j, i : i + XS, 0:ZW]
                else:
                    in0 = xslab[:, :, :, j, i : i + XS, l : l + ZW]
                in1 = (
                    wg[:, kk].unsqueeze(2).broadcast_to([128, NSLOT, B, XS, ZW])
                )
                if on_pool:
                    prod = ppool.tile(
                        [128, NSLOT, B, XS, ZW], f16, name="prodg", tag="prodg",
                        bufs=3,
                    )
                    nc.gpsimd.tensor_tensor(out=prod[:], in0=in0, in1=in1, op=MUL)
                else:
                    prod = ppool.tile(
                        [128, NSLOT, B, XS, ZW], f16, name="prods", tag="prods",
                        bufs=6,
                    )
                    nc.vector.tensor_tensor(out=prod[:], in0=in0, in1=in1, op=MUL)
                mm_consume(prod)

            assert acc_cnt == NTAP

            # ---- drain: PSUM -> fp16 SBUF -> HBM ----
            for ci in range(PCH):
                outsb = ppool.tile(
                    [128, CCH], f16, name="outsb", tag=f"outsb_{ci}", bufs=1
                )
                # drain tails run in parallel on DVE (first 2) and ACT (last 2)
                if ci < 2:
                    nc.vector.tensor_copy(out=outsb[:], in_=psums[ci][:])
                else:
                    nc.scalar.copy(out=outsb[:], in_=psums[ci][:])
                oq = (nc.sync, nc.gpsimd, nc.scalar, nc.sync)[ci]
                oq.dma_start(
                    out=out_d.ap()[:, ci * CCH : (ci + 1) * CCH], in_=outsb[:]
                )

    nc.compile()
    _GRAPH_CACHE[key] = nc
    return nc


def make_in_maps(X, W):
    """Host-side shard prep. X [2,1,96,96,96] f32, W [1,1,96,96,96,27] f32."""
    from numpy.lib.stride_tricks import sliding_window_view

    X = np.asarray(X)
    W = np.asarray(W)
    Xs = X.reshape(B, D, D, D)  # [b, x, y, z]
    # edge padding on all three spatial dims
    Xp = np.pad(Xs, ((0, 0), (1, 1), (1, 1), (1, 1)), mode="edge").astype(F16)
    # windows over (y, z): [b, xp 98, y0 96, z0 73, y' 3, zz 26]
    swv = sliding_window_view(Xp, (KSZ, ZWH), axis=(2, 3))
    W00 = W.reshape(D, D, D, NTAP).astype(F16)  # [x, y, z, t]
    ident = np.eye(128, dtype=F16)
    tap_perm = np.array(TAP_ORDER)

    in_maps = []
    for m in range(NCORES):
        # slab[r=(zq*96+y), b, y', xi, zz] with xi the 14-wide core x window
        arr = swv[:, m * XS : m * XS + XH, :, 0 : 3 * ZW + 1 : ZW]
        # arr: [b, xi 14, y 96, zq 4, y' 3, zz 26] -> [zq, y, b, y', xi, zz]
        slab = np.ascontiguousarray(np.transpose(arr, (3, 2, 0, 4, 1, 5))).reshape(
            NSLOT, 128, B, KSZ, XH, ZWH
        )
        slab = np.ascontiguousarray(np.transpose(slab, (1, 0, 2, 3, 4, 5)))

        wc = W00[m * XS : (m + 1) * XS][..., tap_perm]  # [xo 12, y, z, t]
        wc = wc.reshape(XS, D, ZQ, ZW, NTAP)  # [xo, y, zq, zo, t]
        wc = np.ascontiguousarray(np.transpose(wc, (2, 1, 4, 0, 3))).reshape(
            NSLOT, 128, NTAP, XS, ZW
        )
        wc = np.ascontiguousarray(np.transpose(wc, (1, 2, 0, 3, 4)))

        in_maps.append({"xslab": slab, "w": wc, "ident": ident})
    return in_maps


def kernel(X, W):
    global LAST_RESULT
    from concourse.bass_utils import run_bass_kernel_spmd

    nc = _build_graph()
    in_maps = make_in_maps(X, W)
    trace = bool(int(os.environ.get("ASYM_TRACE", "0")))
    res = run_bass_kernel_spmd(
        nc, in_maps, core_ids=list(range(NCORES)), trace=trace
    )
    LAST_RESULT = res

    out = np.empty((B, 1, D, D, D), dtype=np.float32)
    for m in range(NCORES):
        r = res.results[m]["out"].astype(np.float32)  # [128, 1728]
        r = r.reshape(128, NSLOT, B, XS, ZW)
        r = np.transpose(r, (1, 0, 2, 3, 4)).reshape(ZQ, D, B, XS, ZW)
        # [zq, y, b, xo, zo] -> [b, xo, y, zq, zo]
        r = np.transpose(r, (2, 3, 1, 0, 4)).reshape(B, XS, D, D)
        out[:, 0, m * XS : (m + 1) * XS, :, :] = r
    return out


# revision 14
# speedup vs baseline: 1.1392x; 1.1392x over previous
"""Location-dependent 3D conv (AsymConv) on 8 TRN2 NeuronCores.

Math (per output voxel):
    out[b, 0, x, y, z] = sum_{i,j,l in 0..2} Xp[b, x+i, y+j, z+l] * W[x, y, z, (i*3+j)*3+l]
with Xp = edge-padded X by 1 plane on each spatial side.

Strategy (128-partition z-split layout):
  - Shard the x axis across cores (12 planes each, halo 14) -> no inter-core
    communication.
  - Per core, partition rows r = (zq, y) with zq in 0..3 a z-quarter and y the
    full 96: 384 rows = exactly 3 "slots" of 128 partitions, kept as one
    merged SBUF tile so every DVE op uses all 128 lanes (the old y-partition
    layout only used 96). Batch b lives in the free dims; W (which has no
    batch dim) is read via a stride-0 broadcast AP, so each W element moves
    from HBM exactly once: per-core traffic 5.97 MB W + 0.84 MB X + 0.44 MB
    out ~= 7.3 MB (~20 us at 360 GB/s) vs ~8 MB before.
  - Per partition the host ships a halo slab [b, y' 3, x 14, zw 26]: all 27
    taps are then free-dim offsets (j, i, l) into the slab; no partition-dim
    shifts anywhere.
  - Products run on the Vector engine (tensor_tensor, fp16 2x mode). Taps
    with l in {0, 2} of each (i, j) fuse into one op via an overlapping
    stride-2 AP dim; l == 1 starts are 2B-misaligned (breaks 2x), so those
    taps read a z-shifted slab copy built on the ScalarE. 9 pair ops + 9
    single ops. A few pair units can be offloaded to the otherwise-idle
    GpSimd (Pool) engine to shave the DVE critical path (ASYM_POOL).
  - 27-term accumulation on the TensorEngine: identity[128x128] matmuls into
    4 PSUM chunks of 432 fp32 columns (512-column moving limit, 2KB banks).
  - W moves as 7 grouped DMAs (4 taps each, 2304B descriptors) in consumption
    order; X slabs land first (3 slot DMAs); drain PSUM -> fp16 -> HBM.
"""

import os

import numpy as np

# ---- problem constants (hardcoded per harness rules) ----
B = 2
D = 96  # Dx = Dy = Dz
KSZ = 3
NTAP = KSZ**3  # 27
NCORES = 8
XS = D // NCORES  # 12 x-planes per core
XH = XS + 2  # with halo
NSLOT = 3  # partition tile slots: 384 rows / 128
ZQ = 4  # z quarters
ZW = D // ZQ  # 24 output z per quarter
ZWH = ZW + 2  # z window incl halo
NCOL = NSLOT * B * XS * ZW  # 1728 psum columns
PCH = 4  # psum chunks
CCH = NCOL // PCH  # 432 columns per chunk

F16 = np.float16
LAST_RESULT = None  # BassKernelResults of the most recent run (for test.py)

_GRAPH_CACHE = {}

N_WARMUP = int(os.environ.get("ASYM_WARMUP", "0"))
N_POOL = int(os.environ.get("ASYM_POOL", "0"))  # taps on GpSimd (SBUF-port contention: slows concurrent DVE ops ~4x, keep 0)

# taps with l != 1 are 4B-aligned in the base slab; issue them first so the
# ScalarE z-shift copies (needed by l == 1 taps) are off the critical path.
# Within the l != 1 block consecutive entries are the (l=0, l=2) taps of one
# (i, j), consumed as one fused DVE op.
TAP_ORDER = [
    t
    for lgroup in (False, True)
    for j in range(KSZ)
    for t in range(NTAP)
    if (t % 3 == 1) == lgroup and (t // 3) % 3 == j
]
# W DMA groups (consumption order), 4 taps per transfer (2304B descriptors)
WG_BOUNDS = [0, 2, 4, 8, 12, 16, 20, 24, 27]


def _build_graph():
    """Build (and cache) the per-core Bass graph. Same graph for all 8 cores."""
    key = ("nc", N_WARMUP, N_POOL)
    if key in _GRAPH_CACHE:
        return _GRAPH_CACHE[key]

    from concourse import bacc
    from concourse import bass as _bass
    import concourse.mybir as mybir
    from concourse.tile import TileContext

    f16 = mybir.dt.float16
    f32 = mybir.dt.float32
    MUL = mybir.AluOpType.mult

    nc = bacc.Bacc("TRN2", target_bir_lowering=False, debug=False, num_devices=NCORES)

    xs_d = nc.dram_tensor(
        "xslab", [128, NSLOT, B, KSZ, XH, ZWH], f16, kind="ExternalInput"
    )
    w_d = nc.dram_tensor("w", [128, NTAP, NSLOT, XS, ZW], f16, kind="ExternalInput")
    id_d = nc.dram_tensor("ident", [128, 128], f16, kind="ExternalInput")
    out_d = nc.dram_tensor("out", [128, NCOL], f16, kind="ExternalOutput")

    with TileContext(nc) as tc:
        with (
            tc.tile_pool(name="xp", bufs=1) as xpool,
            tc.tile_pool(name="wp", bufs=1) as wpool,
            tc.tile_pool(name="pp", bufs=4) as ppool,
            tc.tile_pool(name="psp", bufs=1, space="PSUM") as pspool,
        ):
            # ---- DMA in: X slabs first, first W group in parallel on the
            # ACT ring, then the W flood in consumption order ----
            xslab = xpool.tile(
                [128, NSLOT, B, KSZ, XH, ZWH], f16, name="xslab", tag="xslab"
            )
            w_tiles = []  # per group
            for g in range(len(WG_BOUNDS) - 1):
                t0, t1 = WG_BOUNDS[g], WG_BOUNDS[g + 1]
                wg = wpool.tile(
                    [128, t1 - t0, NSLOT, XS, ZW], f16, name=f"w_{g}", tag=f"w_{g}"
                )
                w_tiles.append(wg)

            # Input DMAs lead on the SP ring (earliest post-preamble issue
            # point); the slab moves as ONE 128-descriptor transfer, W in
            # tap-major layout (contiguous G*1728B per-partition runs). The
            # serialized DIRECT2D descriptor-generation cost (~1us per
            # dma_start on the issuing sequencer) is split across the SP and
            # otherwise-idle GpSimd rings.
            # 3 slot transfers: 2184B descriptors move ~4x faster per byte
            # than one merged 6552B-descriptor DMA (dram page splitting)
            nc.sync.dma_start(out=xslab[:, 0:1], in_=xs_d.ap()[:, 0:1])
            nc.sync.dma_start(out=xslab[:, 1:2], in_=xs_d.ap()[:, 1:2])
            nc.sync.dma_start(out=xslab[:, 2:3], in_=xs_d.ap()[:, 2:3])
            nc.sync.dma_start(out=w_tiles[0][:], in_=w_d.ap()[:, 0:2])
            id_t = xpool.tile([128, 128], f16, name="id_t", tag="id_t")
            nc.scalar.dma_start(out=id_t[:], in_=id_d.ap())
            for g in range(1, len(WG_BOUNDS) - 1):
                t0, t1 = WG_BOUNDS[g], WG_BOUNDS[g + 1]
                nc.sync.dma_start(out=w_tiles[g][:], in_=w_d.ap()[:, t0:t1])

            # ---- z-shifted slab for l == 1 taps (2B-aligned reads);
            # copies are EMITTED mid-stream (see tap loop): instructions
            # emitted earlier on other engines can delay the DVE stream start.
            xz = xpool.tile(
                [128, NSLOT, B, KSZ, XH, ZWH - 1], f16, name="xz", tag="xz"
            )

            def emit_xz_copies():
                for s in range(NSLOT):
                    for b in range(B):
                        nc.scalar.copy(
                            out=xz[:, s, b], in_=xslab[:, s, b, :, :, 1:ZWH]
                        )

            # ---- PSUM accumulators ----
            psums = [
                pspool.tile([128, CCH], f32, name=f"ps_{ci}", tag=f"ps_{ci}")
                for ci in range(PCH)
            ]

            if N_WARMUP:
                dummy = ppool.tile([128, CCH], f16, name="warm", tag="warm", bufs=1)
                nc.vector.memset(dummy[:], 0.0)
                ps_w = pspool.tile([128, CCH], f32, name="ps_warm", tag="ps_warm")
                for _ in range(N_WARMUP):
                    nc.tensor.matmul(ps_w[:], id_t[:], dummy[:], start=True, stop=True)

            # ---- product + accumulate stream ----
            # walrus limits every engine to 3 free AP dims, so each tap is
            # one op: in0 [slot*b (fused), x, z], in1 W b-broadcast, out dense.
            # N_POOL taps run on the otherwise-idle GpSimd engine (no 2x mode
            # there, so they can read l==1 straight from the base slab).
            pool_set = set()
            if N_POOL:
                step = NTAP / N_POOL
                pool_set = {int(step * (n + 0.5)) for n in range(N_POOL)}

            acc_cnt = 0  # taps accumulated so far (start/stop flags)

            def mm_consume(prod):
                nonlocal acc_cnt
                pbase = prod[:]
                for ci in range(PCH):
                    rhs = _bass.AP(
                        pbase.tensor,
                        pbase.offset + ci * CCH,
                        [pbase.ap[0], [1, CCH]],
                    )
                    nc.tensor.matmul(
                        psums[ci][:],
                        id_t[:],
                        rhs,
                        start=(acc_cnt == 0),
                        stop=(acc_cnt == NTAP - 1),
                    )
                acc_cnt += 1

            def wg_of(wi):
                for g in range(len(WG_BOUNDS) - 1):
                    if WG_BOUNDS[g] <= wi < WG_BOUNDS[g + 1]:
                        return w_tiles[g], wi - WG_BOUNDS[g]
                raise AssertionError(wi)

            for wi in range(NTAP):
                if wi == 8:
                    emit_xz_copies()
                t = TAP_ORDER[wi]
                i, j, l = t // 9, (t // 3) % 3, t % 3
                wg, kk = wg_of(wi)
                on_pool = wi in pool_set
                if l == 1 and not on_pool:
                    in0 = xz[:, :, :, j, i : i + XS, 0:ZW]
                else:
                    in0 = xslab[:, :, :, j, i : i + XS, l : l + ZW]
                in1 = (
                    wg[:, kk].unsqueeze(2).broadcast_to([128, NSLOT, B, XS, ZW])
                )
                if on_pool:
                    prod = ppool.tile(
                        [128, NSLOT, B, XS, ZW], f16, name="prodg", tag="prodg",
                        bufs=3,
                    )
                    nc.gpsimd.tensor_tensor(out=prod[:], in0=in0, in1=in1, op=MUL)
                else:
                    prod = ppool.tile(
                        [128, NSLOT, B, XS, ZW], f16, name="prods", tag="prods",
                        bufs=6,
                    )
                    nc.vector.tensor_tensor(out=prod[:], in0=in0, in1=in1, op=MUL)
                mm_consume(prod)

            assert acc_cnt == NTAP

            # ---- drain: PSUM -> fp16 SBUF -> HBM ----
            for ci in range(PCH):
                outsb = ppool.tile(
                    [128, CCH], f16, name="outsb", tag=f"outsb_{ci}", bufs=1
                )
                # drain tails run in parallel on DVE (first 2) and ACT (last 2)
                if ci < 2:
                    nc.vector.tensor_copy(out=outsb[:], in_=psums[ci][:])
                else:
                    nc.scalar.copy(out=outsb[:], in_=psums[ci][:])
                oq = (nc.sync, nc.gpsimd, nc.scalar, nc.sync)[ci]
                oq.dma_start(
                    out=out_d.ap()[:, ci * CCH : (ci + 1) * CCH], in_=outsb[:]
                )

    nc.compile()
    _GRAPH_CACHE[key] = nc
    return nc


def make_in_maps(X, W):
    """Host-side shard prep. X [2,1,96,96,96] f32, W [1,1,96,96,96,27] f32."""
    from numpy.lib.stride_tricks import sliding_window_view

    X = np.asarray(X)
    W = np.asarray(W)
    Xs = X.reshape(B, D, D, D)  # [b, x, y, z]
    # edge padding on all three spatial dims
    Xp = np.pad(Xs, ((0, 0), (1, 1), (1, 1), (1, 1)), mode="edge").astype(F16)
    # windows over (y, z): [b, xp 98, y0 96, z0 73, y' 3, zz 26]
    swv = sliding_window_view(Xp, (KSZ, ZWH), axis=(2, 3))
    W00 = W.reshape(D, D, D, NTAP).astype(F16)  # [x, y, z, t]
    ident = np.eye(128, dtype=F16)
    tap_perm = np.array(TAP_ORDER)

    in_maps = []
    for m in range(NCORES):
        # slab[r=(zq*96+y), b, y', xi, zz] with xi the 14-wide core x window
        arr = swv[:, m * XS : m * XS + XH, :, 0 : 3 * ZW + 1 : ZW]
        # arr: [b, xi 14, y 96, zq 4, y' 3, zz 26] -> [zq, y, b, y', xi, zz]
        slab = np.ascontiguousarray(np.transpose(arr, (3, 2, 0, 4, 1, 5))).reshape(
            NSLOT, 128, B, KSZ, XH, ZWH
        )
        slab = np.ascontiguousarray(np.transpose(slab, (1, 0, 2, 3, 4, 5)))

        wc = W00[m * XS : (m + 1) * XS][..., tap_perm]  # [xo 12, y, z, t]
        wc = wc.reshape(XS, D, ZQ, ZW, NTAP)  # [xo, y, zq, zo, t]
        wc = np.ascontiguousarray(np.transpose(wc, (2, 1, 4, 0, 3))).reshape(
            NSLOT, 128, NTAP, XS, ZW
        )
        wc = np.ascontiguousarray(np.transpose(wc, (1, 2, 0, 3, 4)))

        in_maps.append({"xslab": slab, "w": wc, "ident": ident})
    return in_maps


def kernel(X, W):
    global LAST_RESULT
    from concourse.bass_utils import run_bass_kernel_spmd

    nc = _build_graph()
    in_maps = make_in_maps(X, W)
    trace = bool(int(os.environ.get("ASYM_TRACE", "0")))
    res = run_bass_kernel_spmd(
        nc, in_maps, core_ids=list(range(NCORES)), trace=trace
    )
    LAST_RESULT = res

    out = np.empty((B, 1, D, D, D), dtype=np.float32)
    for m in range(NCORES):
        r = res.results[m]["out"].astype(np.float32)  # [128, 1728]
        r = r.reshape(128, NSLOT, B, XS, ZW)
        r = np.transpose(r, (1, 0, 2, 3, 4)).reshape(ZQ, D, B, XS, ZW)
        # [zq, y, b, xo, zo] -> [b, xo, y, zq, zo]
        r = np.transpose(r, (2, 3, 1, 0, 4)).reshape(B, XS, D, D)
        out[:, 0, m * XS : (m + 1) * XS, :, :] = r
    return out
